# revision 6
# baseline (speedup 1.0000x reference)
"""DoReFa binarized 3x3 conv (stride 1, pad 1) on 8 Trainium2 NeuronCores.

Reference computation (forward values only):
    xb = sign(x)                                  # exactly {-1, 0, +1}
    scale[co] = mean(|w[co]|)                     # over (ci, kh, kw)
    wb = scale * sign(w)
    out = conv2d(xb, wb, stride=1, pad=1)         # NCHW / OIHW

Kernel strategy:
  - Data-parallel over batch: 32 images -> 4 per core, weights replicated.
  - Since sign values are exactly representable in bf16, the conv runs as
    bf16 matmuls with fp32 PSUM accumulation; all partial sums are integers
    |s| <= 2304, so the matmul part is EXACT. The per-channel scale is
    factored out and applied in fp32 on the way out of PSUM.
  - Conv-as-9-shifted-matmuls: sign(x) is written into a zero-padded SBUF
    image laid out with a 64-element row stride; each 3x3 tap then reads a
    plain contiguous [128, 512] slice (8 output rows) shifted by
    (kh-1)*64 + (kw-1), contracting ci (2 blocks of 128) on partitions and
    accumulating 18 matmuls into one PSUM bank per 8-row chunk.
"""

import numpy as np

import concourse.bass as bass
import concourse.mybir as mybir
import concourse.tile as tile
from concourse import bacc
from concourse.bass_utils import run_bass_kernel_spmd
from concourse.masks import make_identity

# Problem shapes (hardcoded per contract)
N_CORES = 8
N_FULL = 32
NI = N_FULL // N_CORES  # images per core
C = 256                 # in channels
CO = 256                # out channels
H = W = 56
P = 128
CB = C // P             # ci blocks (2)
OB = CO // P            # co blocks (2)
TAPS = 9
KK = C * TAPS           # 2304 weight elements per out channel

# Padded sign(x) layout: row stride 64, image padded rows 0..57 live at flat
# rows 1..58 (guard rows 0 and 59 stay zero), cols 0..57 used.
Q = 64
NROWS = 60
PLANE = NROWS * Q       # 3840
CHUNK_ROWS = 8
NCHUNK = H // CHUNK_ROWS  # 7
NFREE = CHUNK_ROWS * Q    # 512

F32 = mybir.dt.float32
BF16 = mybir.dt.bfloat16

_CACHED = {}


def _build_program(dump_debug=False):
    nc = bacc.Bacc(
        "TRN2",
        target_bir_lowering=False,
        debug=False,
        enable_asserts=False,
        num_devices=N_CORES,
    )
    x = nc.dram_tensor("x", [NI, C, H, W], F32, kind="ExternalInput")
    w = nc.dram_tensor("weight", [CO, C, 3, 3], F32, kind="ExternalInput")
    out = nc.dram_tensor("out", [NI, CO, H, W], F32, kind="ExternalOutput")
    if dump_debug:
        d_scale = nc.dram_tensor("d_scale", [P, OB], F32, kind="ExternalOutput")
        d_wT = nc.dram_tensor(
            "d_wT", [P, TAPS, OB, CB, P], BF16, kind="ExternalOutput"
        )
        d_xpad = nc.dram_tensor(
            "d_xpad", [NI, CB, P, PLANE], BF16, kind="ExternalOutput"
        )

    with tile.TileContext(nc) as tc:
        with (
            tc.tile_pool(name="consts", bufs=1) as consts,
            tc.tile_pool(name="wprep", bufs=1) as wprep,
            tc.tile_pool(name="xraw", bufs=2) as xraw_pool,
            tc.tile_pool(name="xpad", bufs=NI * CB) as xpad_pool,
            tc.tile_pool(name="osb", bufs=3) as osb_pool,
            tc.tile_pool(name="psum", bufs=4, space="PSUM") as psum_pool,
            tc.tile_pool(name="psum_tr", bufs=2, space="PSUM") as psum_tr_pool,
        ):
            # ---- weight prep (replicated, tiny) ----
            identity = consts.tile([P, P], F32)
            make_identity(nc, identity)

            wraw = wprep.tile([P, OB, KK], F32)  # [co_p, ob, ci*9]
            nc.sync.dma_start(
                wraw[:], w.rearrange("(ob p) ci kh kw -> p ob (ci kh kw)", p=P)
            )

            # scale[co] = mean |w| over 2304, two-stage reduce for precision
            ssum1 = wprep.tile([P, OB, 18], F32)
            ssum = wprep.tile([P, OB], F32)
            scale = wprep.tile([P, OB], F32)
            for ob in range(OB):
                nc.vector.tensor_reduce(
                    ssum1[:, ob],
                    wraw[:, ob].rearrange("p (a b) -> p a b", b=P),
                    axis=mybir.AxisListType.X,
                    op=mybir.AluOpType.add,
                    apply_absolute_value=True,
                )
                nc.vector.tensor_reduce(
                    ssum[:, ob : ob + 1],
                    ssum1[:, ob],
                    axis=mybir.AxisListType.X,
                    op=mybir.AluOpType.add,
                )
            nc.vector.tensor_scalar_mul(scale[:], ssum[:], 1.0 / KK)

            # transpose sign(w): [co, ci] -> [ci, co] per (tap, ob, cib)
            # PE transposes raw fp32 w, then ACT applies Sign on the PSUM
            # result while casting to bf16.
            wT = wprep.tile([P, TAPS, OB, CB, P], BF16)  # [ci_p, tap, ob, cib, co]
            wraw4 = wraw.rearrange("p ob (ci t) -> p ob ci t", t=TAPS)
            for tap in range(TAPS):
                for ob in range(OB):
                    for cib in range(CB):
                        ptr = psum_tr_pool.tile([P, P], F32)
                        src = wraw4[:, ob, cib * P : (cib + 1) * P, tap]
                        nc.tensor.transpose(ptr[:], src, identity[:])
                        nc.scalar.activation(
                            wT[:, tap, ob, cib, :],
                            ptr[:],
                            mybir.ActivationFunctionType.Sign,
                        )

            # ---- activation prep: sign(x) into padded bf16 planes ----
            xp = {}
            for img in range(NI):
                xr = xraw_pool.tile([P, CB, H * W], F32)
                nc.sync.dma_start(
                    xr[:], x[img].rearrange("(cb p) h w -> p cb (h w)", p=P)
                )
                for cib in range(CB):
                    t = xpad_pool.tile([P, PLANE], BF16, tag="xpad")
                    nc.vector.memset(t[:], 0.0)
                    nc.scalar.activation(
                        t.rearrange("p (r q) -> p r q", q=Q)[:, 2 : 2 + H, 1 : 1 + W],
                        xr[:, cib].rearrange("p (h w) -> p h w", w=W),
                        mybir.ActivationFunctionType.Sign,
                    )
                    xp[(img, cib)] = t

            if dump_debug:
                nc.sync.dma_start(d_scale[:], scale[:])
                nc.sync.dma_start(d_wT[:], wT[:])
                for img in range(NI):
                    for cib in range(CB):
                        nc.sync.dma_start(d_xpad[img, cib], xp[(img, cib)][:])

            # ---- conv: 9 shifted matmuls x 2 ci blocks per 8-row chunk ----
            for img in range(NI):
                for ob in range(OB):
                    osb = osb_pool.tile([P, H * W], F32)
                    osb3 = osb.rearrange("p (h w) -> p h w", w=W)
                    for t in range(NCHUNK):
                        ps = psum_pool.tile([P, NFREE], F32)
                        base = (CHUNK_ROWS * t + 2) * Q
                        mm = 0
                        for cib in range(CB):
                            xpt = xp[(img, cib)]
                            for kh in range(3):
                                for kw in range(3):
                                    tap = kh * 3 + kw
                                    off = base + (kh - 1) * Q + (kw - 1)
                                    nc.tensor.matmul(
                                        ps[:],
                                        wT[:, tap, ob, cib, :],
                                        xpt[:, off : off + NFREE],
                                        start=(mm == 0),
                                        stop=(mm == CB * TAPS - 1),
                                    )
                                    mm += 1
                        nc.vector.tensor_scalar_mul(
                            osb3[:, CHUNK_ROWS * t : CHUNK_ROWS * (t + 1), :],
                            ps.rearrange("p (r q) -> p r q", q=Q)[:, :, 1 : 1 + W],
                            scale[:, ob : ob + 1],
                        )
                    nc.sync.dma_start(
                        out[img, ob * P : (ob + 1) * P].rearrange("p h w -> p (h w)"),
                        osb[:],
                    )
    nc.compile()
    return nc


def get_program():
    if "nc" not in _CACHED:
        _CACHED["nc"] = _build_program()
    return _CACHED["nc"]


def kernel(x: np.ndarray, weight: np.ndarray) -> np.ndarray:
    assert x.shape == (N_FULL, C, H, W) and weight.shape == (CO, C, 3, 3)
    nc = get_program()
    x = np.ascontiguousarray(x, dtype=np.float32)
    weight = np.ascontiguousarray(weight, dtype=np.float32)
    in_maps = [
        {"x": x[i * NI : (i + 1) * NI], "weight": weight} for i in range(N_CORES)
    ]
    res = run_bass_kernel_spmd(nc, in_maps, core_ids=list(range(N_CORES)))
    return np.concatenate([r["out"] for r in res.results], axis=0)


# revision 7
# speedup vs baseline: 421.6235x; 421.6235x over previous
"""DoReFa binarized 3x3 conv (stride 1, pad 1) on 8 Trainium2 NeuronCores.

Reference computation (forward values only):
    xb = sign(x)                                  # exactly {-1, 0, +1}
    scale[co] = mean(|w[co]|)                     # over (ci, kh, kw)
    wb = scale * sign(w)
    out = conv2d(xb, wb, stride=1, pad=1)         # NCHW / OIHW

Kernel strategy:
  - Data-parallel over batch: 32 images -> 4 per core, weights replicated.
  - Since sign values are exactly representable in bf16, the conv runs as
    bf16 matmuls with fp32 PSUM accumulation; all partial sums are integers
    |s| <= 2304, so the matmul part is EXACT. The per-channel scale is
    factored out and applied in fp32 on the way out of PSUM.
  - Conv-as-9-shifted-matmuls: sign(x) is written into a zero-padded SBUF
    image laid out with a 64-element row stride; each 3x3 tap then reads a
    plain contiguous [128, 512] slice (8 output rows) shifted by
    (kh-1)*64 + (kw-1), contracting ci (2 blocks of 128) on partitions and
    accumulating 18 matmuls into one PSUM bank per 8-row chunk.
"""

import numpy as np

import concourse.bass as bass
import concourse.mybir as mybir
import concourse.tile as tile
from concourse import bacc
from concourse.bass_utils import run_bass_kernel_spmd
from concourse.masks import make_identity

# Problem shapes (hardcoded per contract)
N_CORES = 8
N_FULL = 32
NI = N_FULL // N_CORES  # images per core
C = 256                 # in channels
CO = 256                # out channels
H = W = 56
P = 128
CB = C // P             # ci blocks (2)
OB = CO // P            # co blocks (2)
TAPS = 9
KK = C * TAPS           # 2304 weight elements per out channel

# Padded sign(x) layout: row stride 64, image padded rows 0..57 live at flat
# rows 1..58 (guard rows 0 and 59 stay zero), cols 0..57 used.
Q = 64
NROWS = 60
PLANE = NROWS * Q       # 3840
CHUNK_ROWS = 8
NCHUNK = H // CHUNK_ROWS  # 7
NFREE = CHUNK_ROWS * Q    # 512

F32 = mybir.dt.float32
BF16 = mybir.dt.bfloat16

_CACHED = {}


def _build_program(dump_debug=False, loop_n=1):
    nc = bacc.Bacc(
        "TRN2",
        target_bir_lowering=False,
        debug=False,
        enable_asserts=False,
        num_devices=N_CORES,
    )
    x = nc.dram_tensor("x", [NI, C, H, W], F32, kind="ExternalInput")
    w = nc.dram_tensor("weight", [CO, C, 3, 3], F32, kind="ExternalInput")
    out = nc.dram_tensor("out", [NI, CO, H, W], F32, kind="ExternalOutput")
    if dump_debug:
        d_scale = nc.dram_tensor("d_scale", [P, OB], F32, kind="ExternalOutput")
        d_wT = nc.dram_tensor(
            "d_wT", [P, TAPS, OB, CB, P], BF16, kind="ExternalOutput"
        )
        d_xpad = nc.dram_tensor(
            "d_xpad", [NI, CB, P, PLANE], BF16, kind="ExternalOutput"
        )

    with tile.TileContext(nc) as tc:
        import contextlib

        loop_ctx = (
            tc.For_i(
                0,
                loop_n,
                1,
                hint_engines=tuple(nc.engines),
            )
            if loop_n > 1
            else contextlib.nullcontext()
        )
        with (
            tc.tile_pool(name="consts", bufs=1) as consts,
            tc.tile_pool(name="wprep", bufs=1) as wprep,
            tc.tile_pool(name="xraw", bufs=2) as xraw_pool,
            tc.tile_pool(name="xpad", bufs=NI * CB) as xpad_pool,
            tc.tile_pool(name="osb", bufs=3) as osb_pool,
            tc.tile_pool(name="psum", bufs=4, space="PSUM") as psum_pool,
            tc.tile_pool(name="psum_tr", bufs=2, space="PSUM") as psum_tr_pool,
            loop_ctx,
        ):
            # ---- weight prep (replicated, tiny) ----
            identity = consts.tile([P, P], F32)
            make_identity(nc, identity)

            wraw = wprep.tile([P, OB, KK], F32)  # [co_p, ob, ci*9]
            nc.sync.dma_start(
                wraw[:], w.rearrange("(ob p) ci kh kw -> p ob (ci kh kw)", p=P)
            )

            # scale[co] = mean |w| over 2304, two-stage reduce for precision
            ssum1 = wprep.tile([P, OB, 18], F32)
            ssum = wprep.tile([P, OB], F32)
            scale = wprep.tile([P, OB], F32)
            for ob in range(OB):
                nc.vector.tensor_reduce(
                    ssum1[:, ob],
                    wraw[:, ob].rearrange("p (a b) -> p a b", b=P),
                    axis=mybir.AxisListType.X,
                    op=mybir.AluOpType.add,
                    apply_absolute_value=True,
                )
                nc.vector.tensor_reduce(
                    ssum[:, ob : ob + 1],
                    ssum1[:, ob],
                    axis=mybir.AxisListType.X,
                    op=mybir.AluOpType.add,
                )
            nc.vector.tensor_scalar_mul(scale[:], ssum[:], 1.0 / KK)

            # transpose sign(w): [co, ci] -> [ci, co] per (tap, ob, cib)
            # PE transposes raw fp32 w, then ACT applies Sign on the PSUM
            # result while casting to bf16.
            wT = wprep.tile([P, TAPS, OB, CB, P], BF16)  # [ci_p, tap, ob, cib, co]
            wraw4 = wraw.rearrange("p ob (ci t) -> p ob ci t", t=TAPS)
            for tap in range(TAPS):
                for ob in range(OB):
                    for cib in range(CB):
                        ptr = psum_tr_pool.tile([P, P], F32)
                        src = wraw4[:, ob, cib * P : (cib + 1) * P, tap]
                        nc.tensor.transpose(ptr[:], src, identity[:])
                        nc.scalar.activation(
                            wT[:, tap, ob, cib, :],
                            ptr[:],
                            mybir.ActivationFunctionType.Sign,
                        )

            # ---- activation prep: sign(x) into padded bf16 planes ----
            xp = {}
            for img in range(NI):
                xr = xraw_pool.tile([P, CB, H * W], F32)
                nc.sync.dma_start(
                    xr[:], x[img].rearrange("(cb p) h w -> p cb (h w)", p=P)
                )
                for cib in range(CB):
                    t = xpad_pool.tile([P, PLANE], BF16, tag="xpad")
                    nc.vector.memset(t[:], 0.0)
                    nc.scalar.activation(
                        t.rearrange("p (r q) -> p r q", q=Q)[:, 2 : 2 + H, 1 : 1 + W],
                        xr[:, cib].rearrange("p (h w) -> p h w", w=W),
                        mybir.ActivationFunctionType.Sign,
                    )
                    xp[(img, cib)] = t

            if dump_debug:
                nc.sync.dma_start(d_scale[:], scale[:])
                nc.sync.dma_start(d_wT[:], wT[:])
                for img in range(NI):
                    for cib in range(CB):
                        nc.sync.dma_start(d_xpad[img, cib], xp[(img, cib)][:])

            # ---- conv: 9 shifted matmuls x 2 ci blocks per 8-row chunk ----
            for img in range(NI):
                for ob in range(OB):
                    osb = osb_pool.tile([P, H * W], F32)
                    osb3 = osb.rearrange("p (h w) -> p h w", w=W)
                    for t in range(NCHUNK):
                        ps = psum_pool.tile([P, NFREE], F32)
                        base = (CHUNK_ROWS * t + 2) * Q
                        mm = 0
                        for cib in range(CB):
                            xpt = xp[(img, cib)]
                            for kh in range(3):
                                for kw in range(3):
                                    tap = kh * 3 + kw
                                    off = base + (kh - 1) * Q + (kw - 1)
                                    nc.tensor.matmul(
                                        ps[:],
                                        wT[:, tap, ob, cib, :],
                                        xpt[:, off : off + NFREE],
                                        start=(mm == 0),
                                        stop=(mm == CB * TAPS - 1),
                                    )
                                    mm += 1
                        nc.vector.tensor_scalar_mul(
                            osb3[:, CHUNK_ROWS * t : CHUNK_ROWS * (t + 1), :],
                            ps.rearrange("p (r q) -> p r q", q=Q)[:, :, 1 : 1 + W],
                            scale[:, ob : ob + 1],
                        )
                    nc.sync.dma_start(
                        out[img, ob * P : (ob + 1) * P].rearrange("p h w -> p (h w)"),
                        osb[:],
                    )
    nc.compile()
    return nc


def get_program():
    if "nc" not in _CACHED:
        _CACHED["nc"] = _build_program()
    return _CACHED["nc"]


def kernel(x: np.ndarray, weight: np.ndarray) -> np.ndarray:
    assert x.shape == (N_FULL, C, H, W) and weight.shape == (CO, C, 3, 3)
    nc = get_program()
    x = np.ascontiguousarray(x, dtype=np.float32)
    weight = np.ascontiguousarray(weight, dtype=np.float32)
    in_maps = [
        {"x": x[i * NI : (i + 1) * NI], "weight": weight} for i in range(N_CORES)
    ]
    res = run_bass_kernel_spmd(nc, in_maps, core_ids=list(range(N_CORES)))
    return np.concatenate([r["out"] for r in res.results], axis=0)


# revision 13
# speedup vs baseline: 599.4364x; 1.4217x over previous
"""DoReFa binarized 3x3 conv (stride 1, pad 1) on 8 Trainium2 NeuronCores.

Reference computation (forward values only):
    xb = sign(x)                                  # exactly {-1, 0, +1}
    scale[co] = mean(|w[co]|)                     # over (ci, kh, kw)
    wb = scale * sign(w)
    out = conv2d(xb, wb, stride=1, pad=1)         # NCHW / OIHW

Kernel strategy:
  - Data-parallel over batch: 32 images -> 4 per core, weights replicated.
  - Since sign values are exactly representable in bf16, the conv runs as
    bf16 matmuls with fp32 PSUM accumulation; all partial sums are integers
    |s| <= 2304, so the matmul part is EXACT. The per-channel scale is
    factored out and applied in fp32 on the way out of PSUM.
  - Conv-as-9-shifted-matmuls: sign(x) is written into a zero-padded SBUF
    image laid out with a 64-element row stride; each 3x3 tap then reads a
    plain contiguous [128, 512] slice (8 output rows) shifted by
    (kh-1)*64 + (kw-1), contracting ci (2 blocks of 128) on partitions and
    accumulating 18 matmuls into one PSUM bank per 8-row chunk.
"""

import numpy as np

import concourse.bass as bass
import concourse.mybir as mybir
import concourse.tile as tile
from concourse import bacc
from concourse.bass_utils import run_bass_kernel_spmd
from concourse.masks import make_identity

# Problem shapes (hardcoded per contract)
N_CORES = 8
N_FULL = 32
NI = N_FULL // N_CORES  # images per core
C = 256                 # in channels
CO = 256                # out channels
H = W = 56
P = 128
CB = C // P             # ci blocks (2)
OB = CO // P            # co blocks (2)
TAPS = 9
KK = C * TAPS           # 2304 weight elements per out channel

# Padded sign(x) layout: row stride 64, image padded rows 0..57 live at flat
# rows 1..58 (guard rows 0 and 59 stay zero), cols 0..57 used.
Q = 64
NROWS = 60
PLANE = NROWS * Q       # 3840
CHUNK_ROWS = 8
NCHUNK = H // CHUNK_ROWS  # 7
NFREE = CHUNK_ROWS * Q    # 512

F32 = mybir.dt.float32
BF16 = mybir.dt.bfloat16
FP8 = mybir.dt.float8e4
BIN_DT = FP8  # dtype for binarized values (sign products are exact)

_CACHED = {}


def _build_program(dump_debug=False, loop_n=1):
    nc = bacc.Bacc(
        "TRN2",
        target_bir_lowering=False,
        debug=False,
        enable_asserts=False,
        num_devices=N_CORES,
    )
    x = nc.dram_tensor("x", [NI, C, H, W], F32, kind="ExternalInput")
    w = nc.dram_tensor("weight", [CO, C, 3, 3], F32, kind="ExternalInput")
    out = nc.dram_tensor("out", [NI, CO, H, W], F32, kind="ExternalOutput")
    if dump_debug:
        d_scale = nc.dram_tensor("d_scale", [P, OB], F32, kind="ExternalOutput")
        d_wT = nc.dram_tensor(
            "d_wT", [P, TAPS, OB, CB, P], BIN_DT, kind="ExternalOutput"
        )
        d_xpad = nc.dram_tensor(
            "d_xpad", [NI, P, CB, PLANE], BIN_DT, kind="ExternalOutput"
        )

    with tile.TileContext(nc) as tc:
        import contextlib

        loop_ctx = (
            tc.For_i(
                0,
                loop_n,
                1,
                hint_engines=tuple(nc.engines),
            )
            if loop_n > 1
            else contextlib.nullcontext()
        )
        with (
            tc.tile_pool(name="consts", bufs=1) as consts,
            tc.tile_pool(name="wprep", bufs=1) as wprep,
            tc.tile_pool(name="xraw", bufs=2) as xraw_pool,
            tc.tile_pool(name="xpad", bufs=NI) as xpad_pool,
            tc.tile_pool(name="osb", bufs=3) as osb_pool,
            tc.tile_pool(name="psum", bufs=4, space="PSUM") as psum_pool,
            tc.tile_pool(name="psum_tr", bufs=2, space="PSUM") as psum_tr_pool,
            loop_ctx,
        ):
            # ---- weight prep (replicated, tiny) ----
            identity = consts.tile([P, P], F32)
            make_identity(nc, identity)

            wraw = wprep.tile([P, OB, KK], F32)  # [co_p, ob, ci*9]
            nc.sync.dma_start(
                wraw[:], w.rearrange("(ob p) ci kh kw -> p ob (ci kh kw)", p=P)
            )

            # scale[co] = mean |w| over 2304, two-stage reduce for precision
            ssum1 = wprep.tile([P, OB, 18], F32)
            ssum = wprep.tile([P, OB], F32)
            scale = wprep.tile([P, OB], F32)
            for ob in range(OB):
                nc.vector.tensor_reduce(
                    ssum1[:, ob],
                    wraw[:, ob].rearrange("p (a b) -> p a b", b=P),
                    axis=mybir.AxisListType.X,
                    op=mybir.AluOpType.add,
                    apply_absolute_value=True,
                )
                nc.vector.tensor_reduce(
                    ssum[:, ob : ob + 1],
                    ssum1[:, ob],
                    axis=mybir.AxisListType.X,
                    op=mybir.AluOpType.add,
                )
            nc.vector.tensor_scalar_mul(scale[:], ssum[:], 1.0 / KK)

            # transpose sign(w): [co, ci] -> [ci, co] per (tap, ob, cib)
            # PE transposes raw fp32 w, then ACT applies Sign on the PSUM
            # result while casting to bf16.
            wT = wprep.tile([P, TAPS, OB, CB, P], BIN_DT)  # [ci_p, tap, ob, cib, co]
            wraw4 = wraw.rearrange("p ob (ci t) -> p ob ci t", t=TAPS)
            for tap in range(TAPS):
                for ob in range(OB):
                    for cib in range(CB):
                        ptr = psum_tr_pool.tile([P, P], F32)
                        src = wraw4[:, ob, cib * P : (cib + 1) * P, tap]
                        nc.tensor.transpose(ptr[:], src, identity[:])
                        nc.scalar.activation(
                            wT[:, tap, ob, cib, :],
                            ptr[:],
                            mybir.ActivationFunctionType.Sign,
                        )

            # ---- activation prep: sign(x) into padded bf16 planes ----
            # one tile per image holding both ci-block planes contiguously so
            # DoubleRow matmuls can address [128, 2, N] with plane stride %16
            xp = {}
            for img in range(NI):
                xr = xraw_pool.tile([P, CB, H * W], F32)
                nc.sync.dma_start(
                    xr[:], x[img].rearrange("(cb p) h w -> p cb (h w)", p=P)
                )
                t = xpad_pool.tile([P, CB, PLANE], BIN_DT, tag="xpad")
                nc.vector.memset(t[:], 0.0)
                for cib in range(CB):
                    nc.scalar.activation(
                        t[:, cib].rearrange("p (r q) -> p r q", q=Q)[
                            :, 2 : 2 + H, 1 : 1 + W
                        ],
                        xr[:, cib].rearrange("p (h w) -> p h w", w=W),
                        mybir.ActivationFunctionType.Sign,
                    )
                xp[img] = t

            if dump_debug:
                nc.sync.dma_start(d_scale[:], scale[:])
                nc.sync.dma_start(d_wT[:], wT[:])
                for img in range(NI):
                    nc.sync.dma_start(d_xpad[img], xp[img][:])

            # ---- conv: 9 DoubleRow matmuls (K=256) per 8-row chunk ----
            for img in range(NI):
                for ob in range(OB):
                    osb = osb_pool.tile([P, H * W], F32)
                    osb3 = osb.rearrange("p (h w) -> p h w", w=W)
                    for t in range(NCHUNK):
                        ps = psum_pool.tile([P, NFREE], F32)
                        base = (CHUNK_ROWS * t + 2) * Q
                        for kh in range(3):
                            for kw in range(3):
                                tap = kh * 3 + kw
                                off = base + (kh - 1) * Q + (kw - 1)
                                nc.tensor.matmul(
                                    ps[:],
                                    wT[:, tap, ob, :, :],
                                    xp[img][:, :, off : off + NFREE],
                                    start=(tap == 0),
                                    stop=(tap == TAPS - 1),
                                    perf_mode=mybir.MatmulPerfMode.DoubleRow,
                                )
                        nc.vector.tensor_scalar_mul(
                            osb3[:, CHUNK_ROWS * t : CHUNK_ROWS * (t + 1), :],
                            ps.rearrange("p (r q) -> p r q", q=Q)[:, :, 1 : 1 + W],
                            scale[:, ob : ob + 1],
                        )
                    nc.sync.dma_start(
                        out[img, ob * P : (ob + 1) * P].rearrange("p h w -> p (h w)"),
                        osb[:],
                    )
    nc.compile()
    return nc


def get_program():
    if "nc" not in _CACHED:
        _CACHED["nc"] = _build_program()
    return _CACHED["nc"]


def kernel(x: np.ndarray, weight: np.ndarray) -> np.ndarray:
    assert x.shape == (N_FULL, C, H, W) and weight.shape == (CO, C, 3, 3)
    nc = get_program()
    x = np.ascontiguousarray(x, dtype=np.float32)
    weight = np.ascontiguousarray(weight, dtype=np.float32)
    in_maps = [
        {"x": x[i * NI : (i + 1) * NI], "weight": weight} for i in range(N_CORES)
    ]
    res = run_bass_kernel_spmd(nc, in_maps, core_ids=list(range(N_CORES)))
    return np.concatenate([r["out"] for r in res.results], axis=0)
